# revision 42
# baseline (speedup 1.0000x reference)
"""Bahdanau additive attention TRN2 Bass kernel.

Shapes (hardcoded): b=8, t_q=32, t_k=1024, n=512, fp32.
Sharding: data-parallel over batch b across the 8 NeuronCores (one batch
element per core).  Weights (Wq, Wk, v_att) are broadcast to every core.

Per-core algorithm (n lives on SBUF partitions, 4 chunks of 128):
  pqT[n, q], pkT[n, k]: projections computed on TensorE after PE-transposing
      query/keys/Wq/Wk (fp32 has no DMA transpose); f32r is used for every
      large matmul (full-rate PE streaming, ~1e-4 rel error).
  scores[q, k] = sum_n v[n] * tanh(pqT[n, q] + pkT[n, k])
      - hybrid tanh evaluation balancing ScalarE vs VectorE:
        * chunk 3: one ACTIVATE per q with the broadcast add fused via the
          per-partition bias operand;
        * chunks 0-2: VectorE tensor_scalar pre-adds (fp32 2x_2p mode) into
          a (128, 4, 1024) batch tile + one bias-free ACTIVATE per
          (chunk, 4 q's), amortizing the ACT per-instruction overhead;
        * the two kinds are interleaved so both engines stay fed.
      - weighted n-partition reduction on TensorE: lhsT = vdiag[c][:, q, :]
        where vdiag[c][p, q', j] = v[c*128+p]*(q'==j), so each matmul adds
        v.tanh into row q of a shared (32, 512) PSUM score tile (one long
        accumulation group per k-half).
  probs = softmax(scores) WITHOUT max subtraction (|scores| <= sum|v| ~ 11,
      fp32-safe): ACTIVATE(Exp, accum_out=rowsum) straight from the PSUM
      score tiles + reciprocal.
  context = (e @ keys) * rsum with e transposed on PE (unnormalized, so the
      transposes don't wait on the normalization).
Returns (context, probs) exactly like the reference.
"""

import numpy as np

B, TQ, TK, N = 8, 32, 1024, 512
P = 128
NCH = N // P   # 4 chunks of n
KCH = TK // P  # 8 chunks of k
KHALF = 2      # t_k split into 2 x 512 for fp32 matmul free-dim limit

# Number of times the whole body is emitted (used by test.py for timing).
# The graded entry point always uses repeat=1.

_nc_cache = {}
_runner_cache = {}


def build_bass(repeat: int = 1, qg: int = 4, fused_first: bool = True):
    import concourse.mybir as mybir
    import concourse.tile as tile
    from concourse import bacc
    from concourse.masks import make_identity

    f32 = mybir.dt.float32
    f32r = mybir.dt.float32r  # same bits as f32; PE streams 1 cyc/row (vs 4)
    AF = mybir.ActivationFunctionType

    # Bacc (not plain Bass): its compile pipeline runs
    # move_matmul_waits_to_ldweights + generate_event_semaphores, which split
    # multi-sem waits to satisfy the 1-wait-per-instruction HW constraint.
    nc = bacc.Bacc()

    q_d = nc.dram_tensor("query_b", [TQ, N], f32, kind="ExternalInput")
    k_d = nc.dram_tensor("keys_b", [TK, N], f32, kind="ExternalInput")
    wq_d = nc.dram_tensor("Wq", [N, N], f32, kind="ExternalInput")
    wk_d = nc.dram_tensor("Wk", [N, N], f32, kind="ExternalInput")
    v_d = nc.dram_tensor("v_att", [N], f32, kind="ExternalInput")
    ctx_d = nc.dram_tensor("context_b", [TQ, N], f32, kind="ExternalOutput")
    probs_d = nc.dram_tensor("probs_b", [TQ, TK], f32, kind="ExternalOutput")

    with tile.TileContext(nc) as tc:
        with (
            tc.tile_pool(name="const", bufs=1) as const,
            tc.tile_pool(name="sbuf", bufs=1) as sbuf,
            tc.tile_pool(name="tanhp", bufs=3) as tanhp,
            tc.tile_pool(name="btp", bufs=2) as btp,
            tc.tile_pool(name="thop", bufs=2) as thop,
            tc.tile_pool(name="psum", bufs=5, space="PSUM") as psum,
            tc.tile_pool(name="psc", bufs=1, space="PSUM") as psc,
        ):
            ident = const.tile([P, P], f32)
            make_identity(nc, ident)

            # v and the vdiag mask tiles are input-constant: build once.
            # vdiag[c][p, q, j] = v[c*P + p] * (q == j); lhsT slice
            # vdiag[c][:, q, :] routes v.tanh into row q of the PSUM score
            # tile while adding zeros to the other 31 rows.
            v_sb = const.tile([P, NCH], f32)
            nc.sync.dma_start(v_sb[:], v_d.rearrange("(o p) -> p o", p=P))
            vdiags = []
            for c in range(NCH):
                vds = const.tile([P, TQ, TQ], f32, tag="vds", name=f"vds{c}")
                nc.vector.memset(vds[:], 1.0)
                nc.vector.tensor_scalar_mul(vds[:], vds[:], v_sb[:, c:c + 1])
                nc.gpsimd.affine_select(
                    out=vds[:], in_=vds[:],
                    pattern=[[1, TQ], [-1, TQ]],
                    compare_op=mybir.AluOpType.is_equal,
                    fill=0.0, base=0, channel_multiplier=0,
                )
                vd = const.tile([P, TQ, TQ], f32r, tag=f"vdiag{c}", name=f"vd{c}")
                nc.vector.tensor_copy(vd[:], vds[:])
                vdiags.append(vd)

            for _ in range(repeat):
                # ---------------- load inputs ----------------
                keys_nat = sbuf.tile([P, KCH, N], f32, tag="keys_nat")
                k_r = k_d.rearrange("(o p) d -> p o d", p=P)
                for ko in range(4):
                    nc.sync.dma_start(keys_nat[:, ko:ko + 1, :], k_r[:, ko:ko + 1, :])
                wk_nat = sbuf.tile([P, NCH, N], f32, tag="wk_nat")
                wk_r = wk_d.rearrange("(o p) d -> p o d", p=P)
                nc.sync.dma_start(wk_nat[:, 0:2, :], wk_r[:, 0:2, :])
                nc.sync.dma_start(wk_nat[:, 2:4, :], wk_r[:, 2:4, :])
                wq_nat = sbuf.tile([P, NCH, N], f32, tag="wq_nat")
                nc.sync.dma_start(wq_nat[:], wq_d.rearrange("(o p) d -> p o d", p=P))
                q_nat = sbuf.tile([TQ, N], f32, tag="q_nat")
                nc.sync.dma_start(q_nat[:], q_d[:])
                nc.sync.dma_start(keys_nat[:, 4:8, :], k_r[:, 4:8, :])

                # ---------------- transposes + projections ----------------
                # Hand-ordered for ramp-up: keysT for the first 4 k-chunks
                # (behind only the first keys DMA), then wkT, then the query
                # path, then the rest of keysT, projections, with pk chunk 3
                # first (it feeds the bias-fused ACT path).
                keysT = sbuf.tile([P, NCH, TK], f32r, tag="keysT")

                def emit_keysT(ko):
                    for do in range(NCH):
                        pt = psum.tile([P, 512], f32, tag="misc", name="pt")
                        nc.tensor.transpose(
                            pt[:, :P], keys_nat[:, ko, do * P:(do + 1) * P], ident[:]
                        )
                        dst = keysT[:, do, ko * P:(ko + 1) * P]
                        if ko >= 4 or do % 2 == 0:
                            nc.scalar.copy(dst, pt[:, :P])
                        else:
                            nc.vector.tensor_copy(dst, pt[:, :P])

                for ko in range(4):
                    emit_keysT(ko)

                wkT = sbuf.tile([P, NCH, N], f32r, tag="wkT")
                for do in range(NCH):
                    for no in range(NCH):
                        pt = psum.tile([P, 512], f32, tag="misc", name="pt")
                        nc.tensor.transpose(
                            pt[:, :P], wk_nat[:, no, do * P:(do + 1) * P], ident[:]
                        )
                        dst = wkT[:, do, no * P:(no + 1) * P]
                        if no % 2 == 0:
                            nc.scalar.copy(dst, pt[:, :P])
                        else:
                            nc.vector.tensor_copy(dst, pt[:, :P])

                # query path
                wqT = sbuf.tile([P, NCH, N], f32, tag="wqT")

                def emit_wqT(no):
                    for do in range(NCH):
                        pt = psum.tile([P, 512], f32, tag="misc", name="pt")
                        nc.tensor.transpose(
                            pt[:, :P], wq_nat[:, no, do * P:(do + 1) * P], ident[:]
                        )
                        nc.vector.tensor_copy(
                            wqT[:, do, no * P:(no + 1) * P], pt[:, :P]
                        )

                emit_wqT(NCH - 1)
                qT = sbuf.tile([P, NCH, TQ], f32, tag="qT")
                for do in range(NCH):
                    pt = psum.tile([P, 512], f32, tag="misc", name="pt")
                    nc.tensor.transpose(
                        pt[:, :TQ], q_nat[:, do * P:(do + 1) * P], ident[:TQ, :TQ]
                    )
                    nc.vector.tensor_copy(qT[:, do, :], pt[:, :TQ])


                # pkT[m, no, k] = pk[k, no*P+m]
                pkT = sbuf.tile([P, NCH, TK], f32, tag="pkT")

                def emit_pk_half(no, kh):
                    pp = psum.tile([P, 512], f32, tag="misc", name="pp")
                    for do in range(NCH):
                        nc.tensor.matmul(
                            pp[:],
                            wkT[:, do, no * P:(no + 1) * P],
                            keysT[:, do, kh * 512:(kh + 1) * 512],
                            start=(do == 0),
                            stop=(do == NCH - 1),
                        )
                    dst = pkT[:, no, kh * 512:(kh + 1) * 512]
                    if no == NCH - 1:
                        nc.scalar.copy(dst, pp[:])
                    else:
                        nc.vector.tensor_copy(dst, pp[:])

                def emit_pk(no):
                    for kh in range(KHALF):
                        emit_pk_half(no, kh)

                # pqT[m, no, q] = pq[q, no*P+m]; only chunk NCH-1 (the
                # bias-fused path's column block) before pk chunk 3, the
                # rest after, so the first fused tanh unblocks sooner.
                pqT = sbuf.tile([P, NCH, TQ], f32, tag="pqT")

                def emit_pq(no):
                    pp = psum.tile([P, 512], f32, tag="misc", name="pp")
                    for do in range(NCH):
                        nc.tensor.matmul(
                            pp[:, :TQ],
                            wqT[:, do, no * P:(no + 1) * P],
                            qT[:, do, :],
                            start=(do == 0),
                            stop=(do == NCH - 1),
                        )
                    nc.vector.tensor_copy(pqT[:, no, :], pp[:, :TQ])

                emit_pq(NCH - 1)
                emit_pk_half(NCH - 1, 0)   # lead fused halves need only this
                for ko in range(4, KCH):
                    emit_keysT(ko)
                emit_pk_half(NCH - 1, 1)
                for no in range(NCH - 1):
                    emit_wqT(no)
                    emit_pq(no)
                    emit_pk(no)

                # ---------------- v-diag mask tiles ----------------
                # vdiag[c][p, q, j] = v[c*P + p] * (q == j); lhsT slice
                # vdiag[c][:, q, :] routes v.tanh into row q of the PSUM
                # score tile while adding zeros to the other 31 rows.
                # ---------------- main tanh / score loop ----------------
                ps_s = [
                    psc.tile([TQ, 512], f32, tag=f"score{h}", name=f"score{h}")
                    for h in range(KHALF)
                ]
                # Hybrid tanh: chunk CSPLIT(=3) uses the bias-fused ACT
                # (emitted first - it only needs pkT[3], so ScalarE has work
                # while the DVE pre-adds ramp); chunks [0, CSPLIT) get DVE
                # pre-adds (fp32 2x_2p) + one batched bias-free ACT per
                # (chunk, 4 q's), which amortizes the ACT per-instruction
                # overhead 4x.  Balances ACT vs DVE.
                CSPLIT = NCH - 1
                QG = qg
                NQG = TQ // QG

                # start flag bookkeeping: the first matmul into each ps_s[h]
                # must carry start=True (k-half-split lead units touch only
                # one h each).
                seen_h = [False, False]

                def score_flags(h, is_last_mm):
                    st = not seen_h[h]
                    seen_h[h] = True
                    return st, is_last_mm

                def emit_fused_half(q, h):
                    th = tanhp.tile([P, 512], f32r, tag="tanh", name="th")
                    nc.scalar.activation(
                        th[:], pkT[:, CSPLIT, h * 512:(h + 1) * 512], AF.Tanh,
                        bias=pqT[:, CSPLIT, q:q + 1], scale=1.0,
                    )
                    st, sp = score_flags(h, False)
                    nc.tensor.matmul(
                        ps_s[h][:, :], vdiags[CSPLIT][:, q, :], th[:],
                        start=st, stop=sp,
                    )

                def emit_fused_one(q, is_last):
                    th = tanhp.tile([P, TK], f32r, tag="tanh", name="th")
                    nc.scalar.activation(
                        th[:], pkT[:, CSPLIT, :], AF.Tanh,
                        bias=pqT[:, CSPLIT, q:q + 1], scale=1.0,
                    )
                    for h in range(KHALF):
                        st, sp = score_flags(h, is_last)
                        nc.tensor.matmul(
                            ps_s[h][:, :],
                            vdiags[CSPLIT][:, q, :],
                            th[:, h * 512:(h + 1) * 512],
                            start=st, stop=sp,
                        )

                def emit_batched_one(c, qgi, is_last):
                    bt = btp.tile([P, QG, TK], f32, tag="bt")
                    for qi in range(QG):
                        q = qgi * QG + qi
                        nc.vector.tensor_scalar_add(
                            bt[:, qi, :], pkT[:, c, :], pqT[:, c, q:q + 1]
                        )
                    tho = thop.tile([P, QG, TK], f32r, tag="tho")
                    nc.scalar.activation(tho[:], bt[:], AF.Tanh)
                    for h in range(KHALF):
                        for qi in range(QG):
                            q = qgi * QG + qi
                            st, sp = score_flags(h, is_last and qi == QG - 1)
                            nc.tensor.matmul(
                                ps_s[h][:, :],
                                vdiags[c][:, q, :],
                                tho[:, qi, h * 512:(h + 1) * 512],
                                start=st, stop=sp,
                            )

                # Interleave fused (ACT-only) and batched (DVE+ACT) units so
                # both engines stay fed.  The LEAD fused units are split by
                # k-half: their first halves depend only on pkT[:,3,:512]
                # (16 keys transposes instead of 32), starting ScalarE's
                # tanh work ~8us earlier.
                NFUSED = TQ // 2
                LEAD = 8
                fused_q = list(range(LEAD, NFUSED))
                batched = [(c, g) for c in range(CSPLIT) for g in range(NQG)]
                batched += [(CSPLIT, g) for g in range(NFUSED // QG, NQG)]
                for q in range(LEAD):
                    emit_fused_half(q, 0)
                for q in range(LEAD):
                    emit_fused_half(q, 1)
                order = []
                fi, bi = 0, 0
                while fi < len(fused_q) or bi < len(batched):
                    if bi < len(batched):
                        order.append(("b", batched[bi])); bi += 1
                    if fi < len(fused_q):
                        order.append(("f", fused_q[fi])); fi += 1
                for idx, (kind, val) in enumerate(order):
                    is_last = idx == len(order) - 1
                    if kind == "f":
                        emit_fused_one(val, is_last)
                    else:
                        emit_batched_one(val[0], val[1], is_last)

                # rounded copy of keys for the f32r context matmul
                keysr = sbuf.tile([P, KCH, N], f32r, tag="keysr")
                nc.vector.tensor_copy(keysr[:], keys_nat[:])

                # ---------------- softmax ----------------
                # |scores| <= sum|v| ~ 11.4, so fp32 exp is safe without the
                # usual max subtraction; read the PSUM score tiles directly.
                e_t = thop.tile([TQ, TK], f32, tag="tho", name="e_t")
                sume = [
                    sbuf.tile([TQ, 1], f32, tag=f"sume{h}", name=f"sume{h}")
                    for h in range(KHALF)
                ]
                for h in range(KHALF):
                    nc.scalar.activation(
                        e_t[:, h * 512:(h + 1) * 512], ps_s[h][:], AF.Exp,
                        accum_out=sume[h][:],
                    )
                sumexp = sbuf.tile([TQ, 1], f32, tag="sumexp")
                nc.vector.tensor_tensor(
                    sumexp[:], sume[0][:], sume[1][:], mybir.AluOpType.add
                )
                eT = sbuf.tile([P, KCH, TQ], f32r, tag="eT")
                rsum = sbuf.tile([TQ, 1], f32, tag="rsum")
                nc.vector.reciprocal(rsum[:], sumexp[:])
                # context accumulation interleaved with the e transposes
                pc = psum.tile([P, 512], f32, tag="misc", name="pc")
                for ko in range(KCH):
                    pt = psum.tile([P, 512], f32, tag="misc", name="pt")
                    nc.tensor.transpose(
                        pt[:, :TQ], e_t[:, ko * P:(ko + 1) * P], ident[:TQ, :TQ]
                    )
                    nc.vector.tensor_copy(eT[:, ko, :], pt[:, :TQ])
                    nc.tensor.matmul(
                        pc[:TQ, :],
                        eT[:, ko, :],
                        keysr[:, ko, :],
                        start=(ko == 0),
                        stop=(ko == KCH - 1),
                    )
                # normalized probs output
                probs = btp.tile([TQ, TK], f32, tag="bt", name="probs")
                nc.vector.tensor_scalar_mul(probs[:], e_t[:], rsum[:])
                nc.sync.dma_start(probs_d[:], probs[:])
                ctx_sb = sbuf.tile([TQ, N], f32, tag="ctx")
                nc.vector.tensor_scalar_mul(ctx_sb[:], pc[:TQ, :], rsum[:])
                nc.sync.dma_start(ctx_d[:], ctx_sb[:])

    nc.finalize()
    return nc


def _get_nc(repeat: int = 1):
    if repeat not in _nc_cache:
        _nc_cache[repeat] = build_bass(repeat=repeat)
    return _nc_cache[repeat]


def _make_runner(nc, n_cores: int):
    """Build a cached jitted shard_map runner for `nc` (axon/PJRT path).

    Mirrors concourse.bass2jax.run_bass_via_pjrt but keeps the compiled
    executable across calls.
    """
    import jax
    import numpy as np
    from jax.sharding import Mesh, PartitionSpec
    from jax.experimental.shard_map import shard_map
    import concourse.mybir as mybir
    from concourse import bass2jax

    bass2jax.install_neuronx_cc_hook()

    partition_name = nc.partition_id_tensor.name if nc.partition_id_tensor else None

    in_names, out_names, out_avals, zero_outs = [], [], [], []
    for alloc in nc.m.functions[0].allocations:
        if not isinstance(alloc, mybir.MemoryLocationSet):
            continue
        name = alloc.memorylocations[0].name
        if alloc.kind == "ExternalInput":
            if name != partition_name:
                in_names.append(name)
        elif alloc.kind == "ExternalOutput":
            out_names.append(name)
            shape = tuple(alloc.tensor_shape)
            dtype = mybir.dt.np(alloc.dtype)
            out_avals.append(jax.core.ShapedArray(shape, dtype))
            zero_outs.append(np.zeros(shape, dtype))
    n_params = len(in_names)
    n_outs = len(out_avals)
    all_in_names = list(in_names) + list(out_names)
    if partition_name is not None:
        all_in_names.append(partition_name)

    donate = tuple(range(n_params, n_params + n_outs))

    def _body(*args):
        operands = list(args)
        if partition_name is not None:
            operands.append(bass2jax.partition_id_tensor())
        outs = bass2jax._bass_exec_p.bind(
            *operands,
            out_avals=tuple(out_avals),
            in_names=tuple(all_in_names),
            out_names=tuple(out_names),
            lowering_input_output_aliases=(),
            sim_require_finite=True,
            sim_require_nnan=True,
            nc=nc,
        )
        return tuple(outs)

    devices = jax.devices()[:n_cores]
    mesh = Mesh(np.asarray(devices), ("core",))
    in_specs = (PartitionSpec("core"),) * (n_params + n_outs)
    out_specs = (PartitionSpec("core"),) * len(out_names)
    sharded = jax.jit(
        shard_map(_body, mesh=mesh, in_specs=in_specs, out_specs=out_specs,
                  check_rep=False),
        donate_argnums=donate,
        keep_unused=True,
    )

    def run(in_maps):
        per_core = [[np.asarray(m[nm]) for nm in in_names] for m in in_maps]
        concat_in = [
            np.concatenate([per_core[c][i] for c in range(n_cores)], axis=0)
            for i in range(n_params)
        ]
        concat_zeros = [
            np.zeros((n_cores * z.shape[0], *z.shape[1:]), z.dtype)
            for z in zero_outs
        ]
        out_arrs = sharded(*concat_in, *concat_zeros)
        return [
            {
                nm: np.asarray(out_arrs[i]).reshape(n_cores, *out_avals[i].shape)[c]
                for i, nm in enumerate(out_names)
            }
            for c in range(n_cores)
        ]

    run.sharded = sharded
    run.in_names = in_names
    run.out_names = out_names
    run.out_avals = out_avals
    run.zero_outs = zero_outs
    run.n_cores = n_cores
    run.mesh = mesh
    return run


def get_runner(repeat: int = 1):
    if repeat not in _runner_cache:
        nc = _get_nc(repeat)
        _runner_cache[repeat] = _make_runner(nc, B)
    return _runner_cache[repeat]


def _in_maps(query, keys, Wq, Wk, v_att):
    query = np.ascontiguousarray(np.asarray(query), dtype=np.float32)
    keys = np.ascontiguousarray(np.asarray(keys), dtype=np.float32)
    Wq = np.ascontiguousarray(np.asarray(Wq), dtype=np.float32)
    Wk = np.ascontiguousarray(np.asarray(Wk), dtype=np.float32)
    v_att = np.ascontiguousarray(np.asarray(v_att), dtype=np.float32)
    return [
        {
            "query_b": query[b],
            "keys_b": keys[b],
            "Wq": Wq,
            "Wk": Wk,
            "v_att": v_att,
        }
        for b in range(B)
    ]


def kernel(query, keys, Wq, Wk, v_att):
    run = get_runner(repeat=1)
    results = run(_in_maps(query, keys, Wq, Wk, v_att))
    context = np.stack([results[b]["context_b"] for b in range(B)])
    probs = np.stack([results[b]["probs_b"] for b in range(B)])
    return context, probs


if __name__ == "__main__":
    rng = np.random.default_rng(0)
    ins = {
        "query": rng.standard_normal((B, TQ, N), dtype=np.float32),
        "keys": rng.standard_normal((B, TK, N), dtype=np.float32),
        "Wq": rng.standard_normal((N, N), dtype=np.float32) / np.sqrt(N),
        "Wk": rng.standard_normal((N, N), dtype=np.float32) / np.sqrt(N),
        "v_att": rng.standard_normal((N,), dtype=np.float32) / np.sqrt(N),
    }
    ctx, pr = kernel(**ins)
    print(ctx.shape, pr.shape, float(np.abs(ctx).max()), float(pr.sum(-1).mean()))


# revision 44
# speedup vs baseline: 1.1468x; 1.1468x over previous
"""Bahdanau additive attention TRN2 Bass kernel.

Shapes (hardcoded): b=8, t_q=32, t_k=1024, n=512, fp32.
Sharding: data-parallel over batch b across the 8 NeuronCores (one batch
element per core).  Weights (Wq, Wk, v_att) are broadcast to every core.

Per-core algorithm (n lives on SBUF partitions, 4 chunks of 128):
  pqT[n, q], pkT[n, k]: projections computed on TensorE after PE-transposing
      query/keys/Wq/Wk (fp32 has no DMA transpose); f32r is used for every
      large matmul (full-rate PE streaming, ~1e-4 rel error).
  scores[q, k] = sum_n v[n] * tanh(pqT[n, q] + pkT[n, k])
      - hybrid tanh evaluation balancing ScalarE vs VectorE:
        * chunk 3: one ACTIVATE per q with the broadcast add fused via the
          per-partition bias operand;
        * chunks 0-2: VectorE tensor_scalar pre-adds (fp32 2x_2p mode) into
          a (128, 4, 1024) batch tile + one bias-free ACTIVATE per
          (chunk, 4 q's), amortizing the ACT per-instruction overhead;
        * the two kinds are interleaved so both engines stay fed.
      - weighted n-partition reduction on TensorE: lhsT = vdiag[c][:, q, :]
        where vdiag[c][p, q', j] = v[c*128+p]*(q'==j), so each matmul adds
        v.tanh into row q of a shared (32, 512) PSUM score tile (one long
        accumulation group per k-half).
  probs = softmax(scores) WITHOUT max subtraction (|scores| <= sum|v| ~ 11,
      fp32-safe): ACTIVATE(Exp, accum_out=rowsum) straight from the PSUM
      score tiles + reciprocal.
  context = (e @ keys) * rsum with e transposed on PE (unnormalized, so the
      transposes don't wait on the normalization).
Returns (context, probs) exactly like the reference.
"""

import numpy as np

B, TQ, TK, N = 8, 32, 1024, 512
P = 128
NCH = N // P   # 4 chunks of n
KCH = TK // P  # 8 chunks of k
KHALF = 2      # t_k split into 2 x 512 for fp32 matmul free-dim limit

# Number of times the whole body is emitted (used by test.py for timing).
# The graded entry point always uses repeat=1.

_nc_cache = {}
_runner_cache = {}


def build_bass(repeat: int = 1, qg: int = 4, fused_first: bool = True):
    import concourse.mybir as mybir
    import concourse.tile as tile
    from concourse import bacc
    from concourse.masks import make_identity

    f32 = mybir.dt.float32
    f32r = mybir.dt.float32r  # same bits as f32; PE streams 1 cyc/row (vs 4)
    AF = mybir.ActivationFunctionType

    # Bacc (not plain Bass): its compile pipeline runs
    # move_matmul_waits_to_ldweights + generate_event_semaphores, which split
    # multi-sem waits to satisfy the 1-wait-per-instruction HW constraint.
    nc = bacc.Bacc()

    q_d = nc.dram_tensor("query_b", [TQ, N], f32, kind="ExternalInput")
    k_d = nc.dram_tensor("keys_b", [TK, N], f32, kind="ExternalInput")
    wq_d = nc.dram_tensor("Wq", [N, N], f32, kind="ExternalInput")
    wk_d = nc.dram_tensor("Wk", [N, N], f32, kind="ExternalInput")
    v_d = nc.dram_tensor("v_att", [N], f32, kind="ExternalInput")
    ctx_d = nc.dram_tensor("context_b", [TQ, N], f32, kind="ExternalOutput")
    probs_d = nc.dram_tensor("probs_b", [TQ, TK], f32, kind="ExternalOutput")

    with tile.TileContext(nc) as tc:
        with (
            tc.tile_pool(name="const", bufs=1) as const,
            tc.tile_pool(name="sbuf", bufs=1) as sbuf,
            tc.tile_pool(name="tanhp", bufs=3) as tanhp,
            tc.tile_pool(name="btp", bufs=2) as btp,
            tc.tile_pool(name="thop", bufs=2) as thop,
            tc.tile_pool(name="psum", bufs=5, space="PSUM") as psum,
            tc.tile_pool(name="psc", bufs=1, space="PSUM") as psc,
        ):
            ident = const.tile([P, P], f32)
            make_identity(nc, ident)

            # v and the vdiag mask tiles are input-constant: build once.
            # vdiag[c][p, q, j] = v[c*P + p] * (q == j); lhsT slice
            # vdiag[c][:, q, :] routes v.tanh into row q of the PSUM score
            # tile while adding zeros to the other 31 rows.
            v_sb = const.tile([P, NCH], f32)
            nc.sync.dma_start(v_sb[:], v_d.rearrange("(o p) -> p o", p=P))
            vdiags = []
            for c in range(NCH):
                vds = const.tile([P, TQ, TQ], f32, tag="vds", name=f"vds{c}")
                nc.vector.memset(vds[:], 1.0)
                nc.vector.tensor_scalar_mul(vds[:], vds[:], v_sb[:, c:c + 1])
                nc.gpsimd.affine_select(
                    out=vds[:], in_=vds[:],
                    pattern=[[1, TQ], [-1, TQ]],
                    compare_op=mybir.AluOpType.is_equal,
                    fill=0.0, base=0, channel_multiplier=0,
                )
                vd = const.tile([P, TQ, TQ], f32r, tag=f"vdiag{c}", name=f"vd{c}")
                nc.vector.tensor_copy(vd[:], vds[:])
                vdiags.append(vd)

            for _ in range(repeat):
                # ---------------- load inputs ----------------
                keys_nat = sbuf.tile([P, KCH, N], f32, tag="keys_nat")
                k_r = k_d.rearrange("(o p) d -> p o d", p=P)
                for ko in range(4):
                    nc.sync.dma_start(keys_nat[:, ko:ko + 1, :], k_r[:, ko:ko + 1, :])
                wk_nat = sbuf.tile([P, NCH, N], f32, tag="wk_nat")
                wk_r = wk_d.rearrange("(o p) d -> p o d", p=P)
                nc.sync.dma_start(wk_nat[:, 0:2, :], wk_r[:, 0:2, :])
                nc.sync.dma_start(wk_nat[:, 2:4, :], wk_r[:, 2:4, :])
                wq_nat = sbuf.tile([P, NCH, N], f32, tag="wq_nat")
                nc.sync.dma_start(wq_nat[:], wq_d.rearrange("(o p) d -> p o d", p=P))
                q_nat = sbuf.tile([TQ, N], f32, tag="q_nat")
                nc.sync.dma_start(q_nat[:], q_d[:])
                nc.sync.dma_start(keys_nat[:, 4:8, :], k_r[:, 4:8, :])

                # ---------------- transposes + projections ----------------
                # Hand-ordered for ramp-up: keysT for the first 4 k-chunks
                # (behind only the first keys DMA), then wkT, then the query
                # path, then the rest of keysT, projections, with pk chunk 3
                # first (it feeds the bias-fused ACT path).
                keysT = sbuf.tile([P, NCH, TK], f32r, tag="keysT")

                def emit_keysT(ko):
                    for do in range(NCH):
                        pt = psum.tile([P, 512], f32, tag="misc", name="pt")
                        nc.tensor.transpose(
                            pt[:, :P], keys_nat[:, ko, do * P:(do + 1) * P], ident[:]
                        )
                        dst = keysT[:, do, ko * P:(ko + 1) * P]
                        if ko >= 4 or do % 2 == 0:
                            nc.scalar.copy(dst, pt[:, :P])
                        else:
                            nc.vector.tensor_copy(dst, pt[:, :P])

                for ko in range(4):
                    emit_keysT(ko)

                wkT = sbuf.tile([P, NCH, N], f32r, tag="wkT")
                for do in range(NCH):
                    for no in range(NCH):
                        pt = psum.tile([P, 512], f32, tag="misc", name="pt")
                        nc.tensor.transpose(
                            pt[:, :P], wk_nat[:, no, do * P:(do + 1) * P], ident[:]
                        )
                        dst = wkT[:, do, no * P:(no + 1) * P]
                        if no % 2 == 0:
                            nc.scalar.copy(dst, pt[:, :P])
                        else:
                            nc.vector.tensor_copy(dst, pt[:, :P])

                # query path
                wqT = sbuf.tile([P, NCH, N], f32, tag="wqT")

                def emit_wqT(no):
                    for do in range(NCH):
                        pt = psum.tile([P, 512], f32, tag="misc", name="pt")
                        nc.tensor.transpose(
                            pt[:, :P], wq_nat[:, no, do * P:(do + 1) * P], ident[:]
                        )
                        nc.vector.tensor_copy(
                            wqT[:, do, no * P:(no + 1) * P], pt[:, :P]
                        )

                emit_wqT(NCH - 1)
                qT = sbuf.tile([P, NCH, TQ], f32, tag="qT")
                for do in range(NCH):
                    pt = psum.tile([P, 512], f32, tag="misc", name="pt")
                    nc.tensor.transpose(
                        pt[:, :TQ], q_nat[:, do * P:(do + 1) * P], ident[:TQ, :TQ]
                    )
                    nc.vector.tensor_copy(qT[:, do, :], pt[:, :TQ])


                # pkT[m, no, k] = pk[k, no*P+m]
                pkT = sbuf.tile([P, NCH, TK], f32, tag="pkT")

                def emit_pk_half(no, kh):
                    pp = psum.tile([P, 512], f32, tag="misc", name="pp")
                    for do in range(NCH):
                        nc.tensor.matmul(
                            pp[:],
                            wkT[:, do, no * P:(no + 1) * P],
                            keysT[:, do, kh * 512:(kh + 1) * 512],
                            start=(do == 0),
                            stop=(do == NCH - 1),
                        )
                    dst = pkT[:, no, kh * 512:(kh + 1) * 512]
                    if no == NCH - 1:
                        nc.scalar.copy(dst, pp[:])
                    else:
                        nc.vector.tensor_copy(dst, pp[:])

                def emit_pk(no):
                    for kh in range(KHALF):
                        emit_pk_half(no, kh)

                # pqT[m, no, q] = pq[q, no*P+m]; only chunk NCH-1 (the
                # bias-fused path's column block) before pk chunk 3, the
                # rest after, so the first fused tanh unblocks sooner.
                pqT = sbuf.tile([P, NCH, TQ], f32, tag="pqT")

                def emit_pq(no):
                    pp = psum.tile([P, 512], f32, tag="misc", name="pp")
                    for do in range(NCH):
                        nc.tensor.matmul(
                            pp[:, :TQ],
                            wqT[:, do, no * P:(no + 1) * P],
                            qT[:, do, :],
                            start=(do == 0),
                            stop=(do == NCH - 1),
                        )
                    nc.vector.tensor_copy(pqT[:, no, :], pp[:, :TQ])

                emit_pq(NCH - 1)
                emit_pk_half(NCH - 1, 0)   # lead fused halves need only this
                for ko in range(4, KCH):
                    emit_keysT(ko)
                emit_pk_half(NCH - 1, 1)
                for no in range(NCH - 1):
                    emit_wqT(no)
                    emit_pq(no)
                    emit_pk(no)

                # ---------------- v-diag mask tiles ----------------
                # vdiag[c][p, q, j] = v[c*P + p] * (q == j); lhsT slice
                # vdiag[c][:, q, :] routes v.tanh into row q of the PSUM
                # score tile while adding zeros to the other 31 rows.
                # ---------------- main tanh / score loop ----------------
                ps_s = [
                    psc.tile([TQ, 512], f32, tag=f"score{h}", name=f"score{h}")
                    for h in range(KHALF)
                ]
                # Hybrid tanh: chunk CSPLIT(=3) uses the bias-fused ACT
                # (emitted first - it only needs pkT[3], so ScalarE has work
                # while the DVE pre-adds ramp); chunks [0, CSPLIT) get DVE
                # pre-adds (fp32 2x_2p) + one batched bias-free ACT per
                # (chunk, 4 q's), which amortizes the ACT per-instruction
                # overhead 4x.  Balances ACT vs DVE.
                CSPLIT = NCH - 1
                QG = qg
                NQG = TQ // QG

                # start flag bookkeeping: the first matmul into each ps_s[h]
                # must carry start=True (k-half-split lead units touch only
                # one h each).
                seen_h = [False, False]

                def score_flags(h, is_last_mm):
                    st = not seen_h[h]
                    seen_h[h] = True
                    return st, is_last_mm

                def emit_fused_half(q, h):
                    th = tanhp.tile([P, 512], f32r, tag="tanh", name="th")
                    nc.scalar.activation(
                        th[:], pkT[:, CSPLIT, h * 512:(h + 1) * 512], AF.Tanh,
                        bias=pqT[:, CSPLIT, q:q + 1], scale=1.0,
                    )
                    st, sp = score_flags(h, False)
                    nc.tensor.matmul(
                        ps_s[h][:, :], vdiags[CSPLIT][:, q, :], th[:],
                        start=st, stop=sp,
                    )

                def emit_fused_one(q, is_last):
                    th = tanhp.tile([P, TK], f32r, tag="tanh", name="th")
                    nc.scalar.activation(
                        th[:], pkT[:, CSPLIT, :], AF.Tanh,
                        bias=pqT[:, CSPLIT, q:q + 1], scale=1.0,
                    )
                    for h in range(KHALF):
                        st, sp = score_flags(h, is_last)
                        nc.tensor.matmul(
                            ps_s[h][:, :],
                            vdiags[CSPLIT][:, q, :],
                            th[:, h * 512:(h + 1) * 512],
                            start=st, stop=sp,
                        )

                def emit_batched_one(c, qgi, is_last):
                    bt = btp.tile([P, QG, TK], f32, tag="bt")
                    for qi in range(QG):
                        q = qgi * QG + qi
                        nc.vector.tensor_scalar_add(
                            bt[:, qi, :], pkT[:, c, :], pqT[:, c, q:q + 1]
                        )
                    tho = thop.tile([P, QG, TK], f32r, tag="tho")
                    nc.scalar.activation(tho[:], bt[:], AF.Tanh)
                    for h in range(KHALF):
                        for qi in range(QG):
                            q = qgi * QG + qi
                            st, sp = score_flags(h, is_last and qi == QG - 1)
                            nc.tensor.matmul(
                                ps_s[h][:, :],
                                vdiags[c][:, q, :],
                                tho[:, qi, h * 512:(h + 1) * 512],
                                start=st, stop=sp,
                            )

                # Interleave fused (ACT-only) and batched (DVE+ACT) units so
                # both engines stay fed.  The LEAD fused units are split by
                # k-half: their first halves depend only on pkT[:,3,:512]
                # (16 keys transposes instead of 32), starting ScalarE's
                # tanh work ~8us earlier.
                NFUSED = TQ // 2
                LEAD = 8
                fused_q = list(range(LEAD, NFUSED))
                batched = [(c, g) for c in range(CSPLIT) for g in range(NQG)]
                batched += [(CSPLIT, g) for g in range(NFUSED // QG, NQG)]
                for q in range(LEAD):
                    emit_fused_half(q, 0)
                for q in range(LEAD):
                    emit_fused_half(q, 1)
                order = []
                fi, bi = 0, 0
                while fi < len(fused_q) or bi < len(batched):
                    if bi < len(batched):
                        order.append(("b", batched[bi])); bi += 1
                    if fi < len(fused_q):
                        order.append(("f", fused_q[fi])); fi += 1
                for idx, (kind, val) in enumerate(order):
                    is_last = idx == len(order) - 1
                    if kind == "f":
                        emit_fused_one(val, is_last)
                    else:
                        emit_batched_one(val[0], val[1], is_last)

                # rounded copy of keys for the f32r context matmul
                keysr = sbuf.tile([P, KCH, N], f32r, tag="keysr")
                nc.vector.tensor_copy(keysr[:], keys_nat[:])

                # ---------------- softmax ----------------
                # |scores| <= sum|v| ~ 11.4, so fp32 exp is safe without the
                # usual max subtraction; read the PSUM score tiles directly.
                e_t = thop.tile([TQ, TK], f32, tag="tho", name="e_t")
                sume = [
                    sbuf.tile([TQ, 1], f32, tag=f"sume{h}", name=f"sume{h}")
                    for h in range(KHALF)
                ]
                for h in range(KHALF):
                    nc.scalar.activation(
                        e_t[:, h * 512:(h + 1) * 512], ps_s[h][:], AF.Exp,
                        accum_out=sume[h][:],
                    )
                sumexp = sbuf.tile([TQ, 1], f32, tag="sumexp")
                nc.vector.tensor_tensor(
                    sumexp[:], sume[0][:], sume[1][:], mybir.AluOpType.add
                )
                eT = sbuf.tile([P, KCH, TQ], f32r, tag="eT")
                rsum = sbuf.tile([TQ, 1], f32, tag="rsum")
                nc.vector.reciprocal(rsum[:], sumexp[:])
                # context accumulation interleaved with the e transposes
                pc = psum.tile([P, 512], f32, tag="misc", name="pc")
                for ko in range(KCH):
                    pt = psum.tile([P, 512], f32, tag="misc", name="pt")
                    nc.tensor.transpose(
                        pt[:, :TQ], e_t[:, ko * P:(ko + 1) * P], ident[:TQ, :TQ]
                    )
                    nc.vector.tensor_copy(eT[:, ko, :], pt[:, :TQ])
                    nc.tensor.matmul(
                        pc[:TQ, :],
                        eT[:, ko, :],
                        keysr[:, ko, :],
                        start=(ko == 0),
                        stop=(ko == KCH - 1),
                    )
                # normalized probs output
                probs = btp.tile([TQ, TK], f32, tag="bt", name="probs")
                nc.vector.tensor_scalar_mul(probs[:], e_t[:], rsum[:])
                nc.sync.dma_start(probs_d[:], probs[:])
                ctx_sb = sbuf.tile([TQ, N], f32, tag="ctx")
                nc.vector.tensor_scalar_mul(ctx_sb[:], pc[:TQ, :], rsum[:])
                nc.sync.dma_start(ctx_d[:], ctx_sb[:])

    nc.finalize()
    return nc


def _get_nc(repeat: int = 1):
    if repeat not in _nc_cache:
        _nc_cache[repeat] = build_bass(repeat=repeat)
    return _nc_cache[repeat]


def _make_runner(nc, n_cores: int):
    """Build a cached jitted shard_map runner for `nc` (axon/PJRT path).

    Mirrors concourse.bass2jax.run_bass_via_pjrt but keeps the compiled
    executable across calls.
    """
    import jax
    import numpy as np
    from jax.sharding import Mesh, PartitionSpec
    from jax.experimental.shard_map import shard_map
    import concourse.mybir as mybir
    from concourse import bass2jax

    bass2jax.install_neuronx_cc_hook()

    partition_name = nc.partition_id_tensor.name if nc.partition_id_tensor else None

    in_names, out_names, out_avals, zero_outs = [], [], [], []
    for alloc in nc.m.functions[0].allocations:
        if not isinstance(alloc, mybir.MemoryLocationSet):
            continue
        name = alloc.memorylocations[0].name
        if alloc.kind == "ExternalInput":
            if name != partition_name:
                in_names.append(name)
        elif alloc.kind == "ExternalOutput":
            out_names.append(name)
            shape = tuple(alloc.tensor_shape)
            dtype = mybir.dt.np(alloc.dtype)
            out_avals.append(jax.core.ShapedArray(shape, dtype))
            zero_outs.append(np.zeros(shape, dtype))
    n_params = len(in_names)
    n_outs = len(out_avals)
    all_in_names = list(in_names) + list(out_names)
    if partition_name is not None:
        all_in_names.append(partition_name)

    donate = tuple(range(n_params, n_params + n_outs))

    def _body(*args):
        operands = list(args)
        if partition_name is not None:
            operands.append(bass2jax.partition_id_tensor())
        outs = bass2jax._bass_exec_p.bind(
            *operands,
            out_avals=tuple(out_avals),
            in_names=tuple(all_in_names),
            out_names=tuple(out_names),
            lowering_input_output_aliases=(),
            sim_require_finite=True,
            sim_require_nnan=True,
            nc=nc,
        )
        return tuple(outs)

    devices = jax.devices()[:n_cores]
    mesh = Mesh(np.asarray(devices), ("core",))
    in_specs = (PartitionSpec("core"),) * (n_params + n_outs)
    out_specs = (PartitionSpec("core"),) * len(out_names)
    sharded = jax.jit(
        shard_map(_body, mesh=mesh, in_specs=in_specs, out_specs=out_specs,
                  check_rep=False),
        donate_argnums=donate,
        keep_unused=True,
    )

    def run(in_maps):
        per_core = [[np.asarray(m[nm]) for nm in in_names] for m in in_maps]
        concat_in = [
            np.concatenate([per_core[c][i] for c in range(n_cores)], axis=0)
            for i in range(n_params)
        ]
        concat_zeros = [
            np.zeros((n_cores * z.shape[0], *z.shape[1:]), z.dtype)
            for z in zero_outs
        ]
        out_arrs = sharded(*concat_in, *concat_zeros)
        return [
            {
                nm: np.asarray(out_arrs[i]).reshape(n_cores, *out_avals[i].shape)[c]
                for i, nm in enumerate(out_names)
            }
            for c in range(n_cores)
        ]

    run.sharded = sharded
    run.in_names = in_names
    run.out_names = out_names
    run.out_avals = out_avals
    run.zero_outs = zero_outs
    run.n_cores = n_cores
    run.mesh = mesh
    return run


def get_runner(repeat: int = 1):
    if repeat not in _runner_cache:
        nc = _get_nc(repeat)
        _runner_cache[repeat] = _make_runner(nc, B)
    return _runner_cache[repeat]


def _in_maps(query, keys, Wq, Wk, v_att):
    query = np.ascontiguousarray(np.asarray(query), dtype=np.float32)
    keys = np.ascontiguousarray(np.asarray(keys), dtype=np.float32)
    Wq = np.ascontiguousarray(np.asarray(Wq), dtype=np.float32)
    Wk = np.ascontiguousarray(np.asarray(Wk), dtype=np.float32)
    v_att = np.ascontiguousarray(np.asarray(v_att), dtype=np.float32)
    return [
        {
            "query_b": query[b],
            "keys_b": keys[b],
            "Wq": Wq,
            "Wk": Wk,
            "v_att": v_att,
        }
        for b in range(B)
    ]


def kernel(query, keys, Wq, Wk, v_att):
    run = get_runner(repeat=1)
    results = run(_in_maps(query, keys, Wq, Wk, v_att))
    context = np.stack([results[b]["context_b"] for b in range(B)])
    probs = np.stack([results[b]["probs_b"] for b in range(B)])
    return context, probs


if __name__ == "__main__":
    rng = np.random.default_rng(0)
    ins = {
        "query": rng.standard_normal((B, TQ, N), dtype=np.float32),
        "keys": rng.standard_normal((B, TK, N), dtype=np.float32),
        "Wq": rng.standard_normal((N, N), dtype=np.float32) / np.sqrt(N),
        "Wk": rng.standard_normal((N, N), dtype=np.float32) / np.sqrt(N),
        "v_att": rng.standard_normal((N,), dtype=np.float32) / np.sqrt(N),
    }
    ctx, pr = kernel(**ins)
    print(ctx.shape, pr.shape, float(np.abs(ctx).max()), float(pr.sum(-1).mean()))
